# revision 48
# baseline (speedup 1.0000x reference)
"""Trainium2 Bass kernel for nn_Bihomogeneous_k3 (bf16 diagonal design).

Math (per batch row, complex z of dim 5 given as z_re/z_im):
  zz[m]   = z_i z_j z_k for the 35 triples i<=j<=k (lexicographic)
  prod    = zz[p] * conj(zz[q]) for the 630 pairs p<=q
  out     = [Re(prod) (630) | Im(prod) strict (595)] -> [B, 1225] f32

Design (pure data parallel over 8 cores, B_local = 16384):
  - Everything on-chip is bf16 (threshold 2e-2 >> bf16's ~2e-3): halves
    output-DMA bytes and enables the DVE 2x_1p perf mode.
  - Feature-major SBUF layout [128 part, c, feat, g]: the batch-row dim g
    is innermost (stride 1) in EVERY access pattern, so all tensor_tensor
    ops (even feature-broadcast ones) qualify for DVE 2x.
  - Products organized by DIAGONAL d = q-p: out_d[t] = zz[t]*conj(zz[t+d])
    keeps both multiplier slices stride-1.  Karatsuba 3-mult per diag:
    m1=R_t R_{t+d}, m2=I_t I_{t+d}, m3=(I_t-R_t)(R_{t+d}+I_{t+d});
    re = m1+m2, im = m3+m1-m2.  m1/m2 computed ONCE per diag and shared
    by the re and im outputs.
  - Device column order (host permutes back during unshard), arranged so
    output chunks complete in creation order:
      [im_1..im_{K-1} | re_0..re_{K-1} | re_K im_K re_{K+1} im_{K+1}...]
    The first K re diags are summed on GPSIMD tensor_tensor straight
    into their output chunks (their m1/m2/m3 mults pinned to DVE so the
    adds start early); everything else accumulates into PSUM via bf16
    +/-identity matmuls (1 cycle/row) filling 16-col windows in column
    order; ACT drains windows (f32 PSUM -> bf16 chunk).  Per-chunk
    writer counting fires each chunk's DMA the moment it completes.
  - Output chunks are 5x256-col tiles (3-buffer rotation); the last two
    overlap by 55 cols (drained twice, same values) so every DMA stays
    >=512 B/row at full descriptor rate.
  - mt1's z/w/zz stage emission is interleaved into mt0's product
    emission (2 closures per diag from d>=K) to hide the mt boundary.
  - Remaining mults greedily balanced DVE (2x packed tensor_tensor,
    0.52 ns/elem) vs GPSIMD tensor_tensor (1.98 ns/elem).
  - Output bf16, diag order; host does out[:, PERM].astype(f32).
  Cost-model result: 223.2 us/core (baseline f32 kernel: 342.9 us) with
  busy/core: PE~152 DVE~144 ACT~138 GPS~138 DMA~119 us; residual idle is
  ~27 us pipeline fill (first zz stage), ~17 us drain+DMA tail, and
  ~20 us scattered buffer-rotation slop.  HW-verified rel err 1.37e-2.
"""
import sys

sys.path.insert(0, "/opt/trn_rl_repo")

import numpy as np

N = 5
NC = 8
B_FULL = 131072
B_LOCAL = B_FULL // NC
P = 128
G = 64
NMT = B_LOCAL // (P * G)  # 2
M = 35
N_RE = 630
N_IM = 595
N_OUT = 1225
WIN = 16
K_GPS = 8  # re diags 0..K-1 summed on GPSIMD (skip PSUM)

# ---- index tables ----
WPAIRS = [(i, j) for i in range(N) for j in range(i, N)]  # 15 lex
WOFF = {}
_o = 0
for (i, j) in WPAIRS:
    WOFF[(i, j)] = _o
    _o += 1
ZOFF = {}
_o = 0
for (i, j) in WPAIRS:
    ZOFF[(i, j)] = _o
    _o += N - j
assert _o == M

# device column layout (diag order), ordered by completion time:
#   im-head  [0, A): im_d for d=1..K-1          (PSUM, drains early)
#   GPS re   [A, B): re_d for d < K             (GPSIMD adds)
#   pairs    [B, N_OUT): re_d, im_d for d >= K  (PSUM)
RE_COL = {}
IM_COL = {}
_c = 0
for d in range(1, K_GPS):
    IM_COL[d] = _c
    _c += M - d
IM_HEAD_END = _c
for d in range(K_GPS):
    RE_COL[d] = _c
    _c += M - d
GPS_END = _c
for d in range(K_GPS, M):
    RE_COL[d] = _c
    _c += M - d
    IM_COL[d] = _c
    _c += M - d
assert _c == N_OUT

# 16-col psum windows over the two PSUM regions
WINDOWS = []
for (_a, _z) in [(0, IM_HEAD_END), (GPS_END, N_OUT)]:
    while _a < _z:
        WINDOWS.append((_a, min(_a + WIN, _z)))
        _a += WIN
import bisect as _bisect
_WSTARTS = [w[0] for w in WINDOWS]


def win_of(col):
    return _bisect.bisect_right(_WSTARTS, col) - 1

# output chunks: 256-col tiles at full DMA rate; the last two chunks
# OVERLAP by 55 cols (drained into both tiles, DMA'd twice with the same
# values) so no chunk drops below the 512-byte full-rate threshold
CHUNKS = [(0, 256), (256, 512), (512, 768), (768, 1024), (969, N_OUT)]
CMAX = 256


def _chunk_of(col):
    for _ci, (_s, _e) in enumerate(CHUNKS):
        if col < _e:
            return _ci
    return len(CHUNKS) - 1

# per-chunk writer counts: gps-add segments + (window x chunk) drain segments
CHUNK_WRITERS = [0] * len(CHUNKS)
for d in range(K_GPS):
    c0, w = RE_COL[d], M - d
    a = c0
    while a < c0 + w:
        ci = _chunk_of(a)
        b = min(CHUNKS[ci][1], c0 + w)
        CHUNK_WRITERS[ci] += 1
        a = b
for (wa, wz) in WINDOWS:
    for ci, (cb, ce) in enumerate(CHUNKS):
        if max(wa, cb) < min(wz, ce):
            CHUNK_WRITERS[ci] += 1

# host-side permutation: lex column j <- device column PERM[j]
PERM = np.zeros(N_OUT, dtype=np.int64)
_c = 0
for p in range(M):
    for q in range(p, M):
        PERM[_c] = RE_COL[q - p] + p
        _c += 1
for p in range(M):
    for q in range(p + 1, M):
        PERM[_c] = IM_COL[q - p] + p
        _c += 1
assert _c == N_OUT


def _ap(base_ap, offset_elems, dims, bassmod):
    return bassmod.AP(tensor=base_ap.tensor, offset=base_ap.offset + offset_elems,
                      ap=[list(base_ap.ap[0])] + [list(d) for d in dims])


def build_bass():
    import concourse.bacc as bacc
    import concourse.bass as bass
    import concourse.tile as tile
    from concourse import mybir
    from contextlib import ExitStack

    f32 = mybir.dt.float32
    bf16 = mybir.dt.bfloat16
    mult = mybir.AluOpType.mult
    add = mybir.AluOpType.add
    sub = mybir.AluOpType.subtract

    nc = bacc.Bacc(None)
    z_re_d = nc.dram_tensor("z_re", [B_LOCAL, N], f32, kind="ExternalInput")
    z_im_d = nc.dram_tensor("z_im", [B_LOCAL, N], f32, kind="ExternalInput")
    ident_d = nc.dram_tensor("ident", [P, P], f32, kind="ExternalInput")
    out_d = nc.dram_tensor("out", [B_LOCAL, N_OUT], bf16, kind="ExternalOutput")

    est = {"v": 0.0, "g": 0.0, "pe": 0.0, "act": 0.0, "dma": 0.0}

    def pick_tt(fd, packed=True, force=None, stt=True):
        # GPS only supports plain TensorTensor (0.42 efficiency ->
        # 1.984 ns/elem); the TensorScalarPtr-with-in1 form fails
        # neuron_isa_check_opcode_on_engine at codegen.
        cv = est["v"] + 60 + (0.521 if packed else 1.042) * fd
        cg = est["g"] + 156 + 1.984 * fd
        if force == "v" or (force is None and cv <= cg):
            est["v"] = cv
            return "v"
        est["g"] = cg
        return "g"

    with tile.TileContext(nc) as tc:
        with ExitStack() as ctx:
            cpool = ctx.enter_context(tc.tile_pool(name="const", bufs=1))
            zpool = ctx.enter_context(tc.tile_pool(name="zp", bufs=1))
            wpool = ctx.enter_context(tc.tile_pool(name="wp", bufs=1))
            zzpool = ctx.enter_context(tc.tile_pool(name="zzp", bufs=2))
            tpool = ctx.enter_context(tc.tile_pool(name="tp", bufs=1))
            mgpool = ctx.enter_context(tc.tile_pool(name="mgp", bufs=2))
            outpool = ctx.enter_context(tc.tile_pool(name="outp", bufs=3))
            pspool = ctx.enter_context(tc.tile_pool(name="ps", bufs=4, space="PSUM"))

            identf = cpool.tile([P, P], f32)
            nc.sync.dma_start(out=identf, in_=ident_d[:, :])
            identP = cpool.tile([P, P], bf16)
            identN = cpool.tile([P, P], bf16)
            nc.scalar.copy(out=identP, in_=identf[:, :])
            nc.scalar.mul(out=identN, in_=identf[:, :], mul=-1.0)
            est["act"] += 2 * (185 + 128 * 0.833)
            idP = identP[:, :]
            idN = identN[:, :]

            warm = pspool.tile([P, WIN * G], f32, tag="ps")
            wrm = warm[:, :]
            nc.tensor.matmul(_ap(wrm, 0, [[1, 1]], bass), idP,
                             _ap(idP, 0, [[1, 1]], bass),
                             start=True, stop=True, skip_group_check=True)
            nc.tensor.matmul(_ap(wrm, 1, [[1, 1]], bass), idN,
                             _ap(idN, 0, [[1, 1]], bass),
                             start=True, stop=True, skip_group_check=True)
            # p-state fillers: keep the PE continuously busy through the
            # z-stage pipeline fill so the first real matmuls run at the
            # ramped 2.4 GHz clock instead of resetting to mid p-state.
            idF = identf[:, :]
            for _f in range(38):
                nc.tensor.matmul(_ap(wrm, 0, [[1, 512]], bass), idF,
                                 _ap(idF, 0, [[1, 512]], bass),
                                 start=True, stop=True, skip_group_check=True)

            def tt(eng, out, in0, in1, op, stt=True):
                if eng == "v":
                    nc.vector.tensor_tensor(out=out, in0=in0, in1=in1, op=op)
                else:
                    nc.gpsimd.tensor_tensor(out=out, in0=in0, in1=in1, op=op)

            cZ, cW, cA = N * G, len(WPAIRS) * G, M * G

            def z_stage_ops(mt):
                """Return (zzT, [closures]) — closures emit the z/w/zz ops."""
                r0 = mt * P * G
                zzT = zzpool.tile([P, 4, M, G], bf16, name=f"zzT{mt}")
                ops = []

                def load():
                    blob = zpool.tile([P, 2, G, N], f32, tag="blob",
                                      name=f"blob{mt}")
                    src_re = z_re_d[r0:r0 + P * G, :].rearrange(
                        "(p g) f -> p g f", g=G)
                    src_im = z_im_d[r0:r0 + P * G, :].rearrange(
                        "(p g) f -> p g f", g=G)
                    nc.sync.dma_start(out=blob[:, 0, :, :], in_=src_re)
                    nc.sync.dma_start(out=blob[:, 1, :, :], in_=src_im)
                    est["dma"] += 2 * 0.46
                    zT = zpool.tile([P, 2, N, G], bf16, tag="zT", name=f"zT{mt}")
                    zb = zT[:, :, :, :]
                    bb = blob[:, :, :, :]
                    nc.scalar.copy(
                        out=_ap(zb, 0, [[cZ, 2], [G, N], [1, G]], bass),
                        in_=_ap(bb, 0, [[G * N, 2], [1, N], [N, G]], bass))
                    est["act"] += 185 + 2 * N * G * 0.833
                    wT = wpool.tile([P, 2, len(WPAIRS), G], bf16, name=f"wT{mt}")
                    state["zb"] = zb
                    state["wbb"] = wT[:, :, :, :]

                state = {}
                ops.append(load)

                def w_step(i):
                    # cross[c1,c2,j] = z[c1,i] * z[c2,i+j]; then
                    # w_re = cross[0,0]-cross[1,1], w_im = cross[1,0]+cross[0,1]
                    zb, wbb = state["zb"], state["wbb"]
                    ti = N - i
                    off = WOFF[(i, i)]
                    t1 = tpool.tile([P, 2, 2, N, G], bf16, tag="ta",
                                    name=f"t1_{mt}_{i}")
                    t1b = t1[:, :, :, :, :]
                    in0 = _ap(zb, i * G, [[cZ, 2], [0, 2], [0, ti], [1, G]], bass)
                    in1 = _ap(zb, i * G, [[0, 2], [cZ, 2], [G, ti], [1, G]], bass)
                    o1 = _ap(t1b, 0,
                             [[2 * N * G, 2], [N * G, 2], [G, ti], [1, G]], bass)
                    tt(pick_tt(4 * ti * G), o1, in0, in1, mult)
                    a0 = _ap(t1b, 0, [[G, ti], [1, G]], bass)
                    a1 = _ap(t1b, 3 * N * G, [[G, ti], [1, G]], bass)
                    ow = _ap(wbb, off * G, [[G, ti], [1, G]], bass)
                    tt(pick_tt(ti * G), ow, a0, a1, sub)
                    a0 = _ap(t1b, 2 * N * G, [[G, ti], [1, G]], bass)
                    a1 = _ap(t1b, N * G, [[G, ti], [1, G]], bass)
                    ow = _ap(wbb, cW + off * G, [[G, ti], [1, G]], bass)
                    tt(pick_tt(ti * G), ow, a0, a1, add)

                for i in range(N):
                    ops.append(lambda i=i: w_step(i))

                ab = zzT[:, :, :, :]

                def zz_step(i, j):
                    # cross[c1,c2,k] = w[c1,(i,j)] * z[c2,j+k]; then
                    # zzR = cross[0,0]-cross[1,1], zzI = cross[1,0]+cross[0,1]
                    zb, wbb = state["zb"], state["wbb"]
                    tk = N - j
                    pr = WOFF[(i, j)]
                    zo = ZOFF[(i, j)]
                    t3 = tpool.tile([P, 2, 2, N, G], bf16, tag="ta",
                                    name=f"t3_{mt}_{pr}")
                    t3b = t3[:, :, :, :, :]
                    in0 = _ap(wbb, pr * G, [[cW, 2], [0, 2], [0, tk], [1, G]], bass)
                    in1 = _ap(zb, j * G, [[0, 2], [cZ, 2], [G, tk], [1, G]], bass)
                    o3 = _ap(t3b, 0,
                             [[2 * N * G, 2], [N * G, 2], [G, tk], [1, G]], bass)
                    tt(pick_tt(4 * tk * G), o3, in0, in1, mult)
                    a0 = _ap(t3b, 0, [[G, tk], [1, G]], bass)
                    a1 = _ap(t3b, 3 * N * G, [[G, tk], [1, G]], bass)
                    oz = _ap(ab, zo * G, [[G, tk], [1, G]], bass)
                    tt(pick_tt(tk * G), oz, a0, a1, sub)
                    a0 = _ap(t3b, 2 * N * G, [[G, tk], [1, G]], bass)
                    a1 = _ap(t3b, N * G, [[G, tk], [1, G]], bass)
                    oz = _ap(ab, cA + zo * G, [[G, tk], [1, G]], bass)
                    tt(pick_tt(tk * G), oz, a0, a1, add)

                for (i, j) in WPAIRS:
                    ops.append(lambda i=i, j=j: zz_step(i, j))

                def sumdif():
                    aR = _ap(ab, 0, [[G, M], [1, G]], bass)
                    aI = _ap(ab, cA, [[G, M], [1, G]], bass)
                    tt(pick_tt(M * G), _ap(ab, 2 * cA, [[G, M], [1, G]], bass),
                       aR, aI, add)
                    tt(pick_tt(M * G), _ap(ab, 3 * cA, [[G, M], [1, G]], bass),
                       aI, aR, sub)

                ops.append(sumdif)
                return zzT, ops

            def products(mt, ab, deferred):
                """Emit product stage for megatile mt; pop some deferred
                z-stage closures (next mt) after each diag."""
                r0 = mt * P * G
                chunk_tiles = {}

                def get_chunk(ci):
                    if ci not in chunk_tiles:
                        occ_t = outpool.tile([P, G, CMAX], bf16, tag="oc",
                                             name=f"oc{mt}_{ci}")
                        chunk_tiles[ci] = occ_t
                    return chunk_tiles[ci]

                writers_left = list(CHUNK_WRITERS)

                def chunk_written(ci):
                    writers_left[ci] -= 1
                    assert writers_left[ci] >= 0
                    if writers_left[ci] == 0:
                        finish_chunk(ci)

                def finish_chunk(ci):
                    cb, ce = CHUNKS[ci]
                    occ = chunk_tiles[ci][:, :, :]
                    dst = out_d[r0:r0 + P * G, cb:ce].rearrange(
                        "(p g) f -> p g f", g=G)
                    nc.sync.dma_start(
                        out=dst, in_=_ap(occ, 0, [[CMAX, G], [1, ce - cb]], bass))
                    est["dma"] += P * G * (ce - cb) * 2 / 360.0

                ps_tiles = {}

                def get_ps(k):
                    if k not in ps_tiles:
                        ps_t = pspool.tile([P, WIN * G], f32, tag="ps",
                                           name=f"ps{mt}_{k}")
                        ps_tiles[k] = ps_t
                    return ps_tiles[k]

                def drain_upto(col):
                    for k, (wa, wz) in enumerate(WINDOWS):
                        if k not in ps_tiles or wz > col:
                            continue
                        psb = ps_tiles.pop(k)[:, :]
                        for ci, (cb, ce) in enumerate(CHUNKS):
                            sa, sz = max(wa, cb), min(wz, ce)
                            if sa >= sz:
                                continue
                            n = sz - sa
                            occ = get_chunk(ci)[:, :, :]
                            src = _ap(psb, (sa - wa) * G, [[G, n], [1, G]], bass)
                            dstc = _ap(occ, sa - cb, [[1, n], [CMAX, G]], bass)
                            nc.scalar.copy(out=dstc, in_=src)
                            est["act"] += 185 + n * G * 0.833
                            chunk_written(ci)

                def pe_accumulate(kind, d, cs, w, mgb):
                    ca = cs
                    while ca < cs + w:
                        k = win_of(ca)
                        wa = WINDOWS[k][0]
                        cb_ = min(ca + 8 - (ca - wa) % 8,
                                  WINDOWS[k][1], cs + w)
                        n = (cb_ - ca) * G
                        psb = get_ps(k)[:, :]
                        pso = _ap(psb, (ca - wa) * G, [[1, n]], bass)
                        t0 = ca - cs
                        m1 = _ap(mgb, t0 * G, [[1, n]], bass)
                        m2 = _ap(mgb, M * G + t0 * G, [[1, n]], bass)
                        if kind == "re":
                            nc.tensor.matmul(pso, idP, m1, start=True, stop=False,
                                             skip_group_check=True)
                            nc.tensor.matmul(pso, idP, m2, start=False, stop=True,
                                             skip_group_check=True)
                            est["pe"] += 2 * n * 0.4167
                        else:
                            m3 = _ap(mgb, 2 * M * G + t0 * G, [[1, n]], bass)
                            nc.tensor.matmul(pso, idP, m3, start=True, stop=False,
                                             skip_group_check=True)
                            nc.tensor.matmul(pso, idP, m1, start=False, stop=False,
                                             skip_group_check=True)
                            nc.tensor.matmul(pso, idN, m2, start=False, stop=True,
                                             skip_group_check=True)
                            est["pe"] += 3 * n * 0.4167
                        ca = cb_

                def gps_add(d, w, mgb):
                    # re_d -> chunk tile(s) on GPSIMD (split at chunk bounds)
                    c0 = RE_COL[d]
                    a = c0
                    while a < c0 + w:
                        ci = _chunk_of(a)
                        cb, ce = CHUNKS[ci]
                        b = min(ce, c0 + w)
                        wl = b - a
                        occ = get_chunk(ci)[:, :, :]
                        og = _ap(occ, a - cb, [[1, wl], [CMAX, G]], bass)
                        m1 = _ap(mgb, (a - c0) * G, [[G, wl], [1, G]], bass)
                        m2 = _ap(mgb, M * G + (a - c0) * G, [[G, wl], [1, G]], bass)
                        nc.gpsimd.tensor_tensor(out=og, in0=m1, in1=m2, op=add)
                        est["g"] += 156 + 1.984 * wl * G
                        chunk_written(ci)
                        a = b

                for d in range(M):
                    w = M - d
                    mg = mgpool.tile([P, 3, M, G], bf16, tag="mg",
                                     name=f"mg{mt}_{d}")
                    mgb = mg[:, :, :, :]
                    in0 = _ap(ab, 0, [[cA, 2], [G, w], [1, G]], bass)
                    in1 = _ap(ab, d * G, [[cA, 2], [G, w], [1, G]], bass)
                    om = _ap(mgb, 0, [[M * G, 2], [G, w], [1, G]], bass)
                    tt(pick_tt(2 * w * G, force="v" if d < K_GPS else None),
                       om, in0, in1, mult)
                    if d >= 1:
                        iD = _ap(ab, 3 * cA, [[G, w], [1, G]], bass)
                        iS = _ap(ab, 2 * cA + d * G, [[G, w], [1, G]], bass)
                        o3 = _ap(mgb, 2 * M * G, [[G, w], [1, G]], bass)
                        tt(pick_tt(w * G, force="v" if d < K_GPS else None),
                           o3, iD, iS, mult)
                    if d < K_GPS:
                        if d >= 1:
                            pe_accumulate("im", d, IM_COL[d], w, mgb)
                        gps_add(d, w, mgb)
                        if d >= 1:
                            drain_upto(IM_COL[d] + w)
                    else:
                        pe_accumulate("re", d, RE_COL[d], w, mgb)
                        pe_accumulate("im", d, IM_COL[d], w, mgb)
                        drain_upto(IM_COL[d] + w)
                    # interleave deferred z-stage emission (next megatile);
                    # hold off until the GPS adds are queued so Pool drains
                    # chunk 0 early
                    if d >= K_GPS:
                        for _ in range(2):
                            if deferred:
                                deferred.pop(0)()
                drain_upto(N_OUT + 1)
                while deferred:
                    deferred.pop(0)()

            zzT0, ops0 = z_stage_ops(0)
            for op in ops0:
                op()
            zzT1, ops1 = z_stage_ops(1)
            products(0, zzT0[:, :, :, :], ops1)
            # boundary fillers: keep the PE ramped across the megatile gap
            warm2 = pspool.tile([P, WIN * G], f32, tag="ps")
            w2b = warm2[:, :]
            for _f in range(30):
                nc.tensor.matmul(_ap(w2b, 0, [[1, 512]], bass), idP,
                                 _ap(idP, 0, [[1, 512]], bass),
                                 start=True, stop=True, skip_group_check=True)
            products(1, zzT1[:, :, :, :], [])

    nc.finalize()
    print(f"[build est us/core] DVE={est['v']/1000:.1f} GPS={est['g']/1000:.1f} "
          f"PE={est['pe']/1000:.1f} ACT={est['act']/1000:.1f} "
          f"DMA={est['dma']/1000:.1f}")
    return nc


_CACHED = {}


def _get_nc():
    if "nc" not in _CACHED:
        _CACHED["nc"] = build_bass()
    return _CACHED["nc"]


def kernel(z_re, z_im):
    from concourse.bass_utils import run_bass_kernel_spmd

    z_re = np.ascontiguousarray(np.asarray(z_re, dtype=np.float32))
    z_im = np.ascontiguousarray(np.asarray(z_im, dtype=np.float32))
    assert z_re.shape == (B_FULL, N), z_re.shape

    nc = _get_nc()
    ident = np.eye(P, dtype=np.float32)
    in_maps = []
    for c in range(NC):
        sl = slice(c * B_LOCAL, (c + 1) * B_LOCAL)
        in_maps.append({
            "z_re": np.ascontiguousarray(z_re[sl]),
            "z_im": np.ascontiguousarray(z_im[sl]),
            "ident": ident,
        })
    res = run_bass_kernel_spmd(nc, in_maps, core_ids=list(range(NC)))
    parts = [np.asarray(res.results[c]["out"]) for c in range(NC)]
    full = np.concatenate(parts, axis=0).astype(np.float32)
    return full[:, PERM]


# revision 49
# speedup vs baseline: 1.1351x; 1.1351x over previous
"""Trainium2 Bass kernel for nn_Bihomogeneous_k3 (bf16 diagonal design).

Math (per batch row, complex z of dim 5 given as z_re/z_im):
  zz[m]   = z_i z_j z_k for the 35 triples i<=j<=k (lexicographic)
  prod    = zz[p] * conj(zz[q]) for the 630 pairs p<=q
  out     = [Re(prod) (630) | Im(prod) strict (595)] -> [B, 1225] f32

Design (pure data parallel over 8 cores, B_local = 16384):
  - Everything on-chip is bf16 (threshold 2e-2 >> bf16's ~2e-3): halves
    output-DMA bytes and enables the DVE 2x_1p perf mode.
  - Feature-major SBUF layout [128 part, c, feat, g]: the batch-row dim g
    is innermost (stride 1) in EVERY access pattern, so all tensor_tensor
    ops (even feature-broadcast ones) qualify for DVE 2x.
  - Products organized by DIAGONAL d = q-p: out_d[t] = zz[t]*conj(zz[t+d])
    keeps both multiplier slices stride-1.  Karatsuba 3-mult per diag:
    m1=R_t R_{t+d}, m2=I_t I_{t+d}, m3=(I_t-R_t)(R_{t+d}+I_{t+d});
    re = m1+m2, im = m3+m1-m2.  m1/m2 computed ONCE per diag and shared
    by the re and im outputs.
  - Device column order (host permutes back during unshard), arranged so
    output chunks complete in creation order:
      [im_1..im_{K-1} | re_0..re_{K-1} | re_K im_K re_{K+1} im_{K+1}...]
    The first K re diags are summed on GPSIMD tensor_tensor straight
    into their output chunks (their m1/m2/m3 mults pinned to DVE so the
    adds start early); everything else accumulates into PSUM via bf16
    +/-identity matmuls (1 cycle/row) filling 16-col windows in column
    order; ACT drains windows (f32 PSUM -> bf16 chunk).  Per-chunk
    writer counting fires each chunk's DMA the moment it completes.
  - Output chunks are 5x256-col tiles (3-buffer rotation); the last two
    overlap by 55 cols (drained twice, same values) so every DMA stays
    >=512 B/row at full descriptor rate.
  - mt1's z/w/zz stage emission is interleaved into mt0's product
    emission (2 closures per diag from d>=K) to hide the mt boundary.
  - Remaining mults greedily balanced DVE (2x packed tensor_tensor,
    0.52 ns/elem) vs GPSIMD tensor_tensor (1.98 ns/elem).
  - Output bf16, diag order; host does out[:, PERM].astype(f32).
  Cost-model result: 223.2 us/core (baseline f32 kernel: 342.9 us) with
  busy/core: PE~152 DVE~144 ACT~138 GPS~138 DMA~119 us; residual idle is
  ~27 us pipeline fill (first zz stage), ~17 us drain+DMA tail, and
  ~20 us scattered buffer-rotation slop.  HW-verified rel err 1.37e-2.
"""
import sys

sys.path.insert(0, "/opt/trn_rl_repo")

import numpy as np

N = 5
NC = 8
B_FULL = 131072
B_LOCAL = B_FULL // NC
P = 128
G = 64
NMT = B_LOCAL // (P * G)  # 2
M = 35
N_RE = 630
N_IM = 595
N_OUT = 1225
WIN = 16
K_GPS = 8  # re diags 0..K-1 summed on GPSIMD (skip PSUM)

# ---- index tables ----
WPAIRS = [(i, j) for i in range(N) for j in range(i, N)]  # 15 lex
WOFF = {}
_o = 0
for (i, j) in WPAIRS:
    WOFF[(i, j)] = _o
    _o += 1
ZOFF = {}
_o = 0
for (i, j) in WPAIRS:
    ZOFF[(i, j)] = _o
    _o += N - j
assert _o == M

# device column layout (diag order), ordered by completion time:
#   im-head  [0, A): im_d for d=1..K-1          (PSUM, drains early)
#   GPS re   [A, B): re_d for d < K             (GPSIMD adds)
#   pairs    [B, N_OUT): re_d, im_d for d >= K  (PSUM)
RE_COL = {}
IM_COL = {}
_c = 0
for d in range(1, K_GPS):
    IM_COL[d] = _c
    _c += M - d
IM_HEAD_END = _c
for d in range(K_GPS):
    RE_COL[d] = _c
    _c += M - d
GPS_END = _c
for d in range(K_GPS, M):
    RE_COL[d] = _c
    _c += M - d
    IM_COL[d] = _c
    _c += M - d
assert _c == N_OUT

# 16-col psum windows over the two PSUM regions
WINDOWS = []
for (_a, _z) in [(0, IM_HEAD_END), (GPS_END, N_OUT)]:
    while _a < _z:
        WINDOWS.append((_a, min(_a + WIN, _z)))
        _a += WIN
import bisect as _bisect
_WSTARTS = [w[0] for w in WINDOWS]


def win_of(col):
    return _bisect.bisect_right(_WSTARTS, col) - 1

# output chunks: 256-col tiles at full DMA rate; the last two chunks
# OVERLAP by 55 cols (drained into both tiles, DMA'd twice with the same
# values) so no chunk drops below the 512-byte full-rate threshold
CHUNKS = [(0, 256), (256, 512), (512, 768), (768, 1024), (969, N_OUT)]
CMAX = 256


def _chunk_of(col):
    for _ci, (_s, _e) in enumerate(CHUNKS):
        if col < _e:
            return _ci
    return len(CHUNKS) - 1

# per-chunk writer counts: gps-add segments + (window x chunk) drain segments
CHUNK_WRITERS = [0] * len(CHUNKS)
for d in range(K_GPS):
    c0, w = RE_COL[d], M - d
    a = c0
    while a < c0 + w:
        ci = _chunk_of(a)
        b = min(CHUNKS[ci][1], c0 + w)
        CHUNK_WRITERS[ci] += 1
        a = b
for (wa, wz) in WINDOWS:
    for ci, (cb, ce) in enumerate(CHUNKS):
        if max(wa, cb) < min(wz, ce):
            CHUNK_WRITERS[ci] += 1

# host-side permutation: lex column j <- device column PERM[j]
PERM = np.zeros(N_OUT, dtype=np.int64)
_c = 0
for p in range(M):
    for q in range(p, M):
        PERM[_c] = RE_COL[q - p] + p
        _c += 1
for p in range(M):
    for q in range(p + 1, M):
        PERM[_c] = IM_COL[q - p] + p
        _c += 1
assert _c == N_OUT


def _ap(base_ap, offset_elems, dims, bassmod):
    return bassmod.AP(tensor=base_ap.tensor, offset=base_ap.offset + offset_elems,
                      ap=[list(base_ap.ap[0])] + [list(d) for d in dims])


def build_bass():
    import concourse.bacc as bacc
    import concourse.bass as bass
    import concourse.tile as tile
    from concourse import mybir
    from contextlib import ExitStack

    f32 = mybir.dt.float32
    bf16 = mybir.dt.bfloat16
    mult = mybir.AluOpType.mult
    add = mybir.AluOpType.add
    sub = mybir.AluOpType.subtract

    nc = bacc.Bacc(None)
    z_re_d = nc.dram_tensor("z_re", [B_LOCAL, N], f32, kind="ExternalInput")
    z_im_d = nc.dram_tensor("z_im", [B_LOCAL, N], f32, kind="ExternalInput")
    ident_d = nc.dram_tensor("ident", [P, P], f32, kind="ExternalInput")
    out_d = nc.dram_tensor("out", [B_LOCAL, N_OUT], bf16, kind="ExternalOutput")

    est = {"v": 0.0, "g": 0.0, "pe": 0.0, "act": 0.0, "dma": 0.0}

    def pick_tt(fd, packed=True, force=None, stt=True):
        # GPS only supports plain TensorTensor (0.42 efficiency ->
        # 1.984 ns/elem); the TensorScalarPtr-with-in1 form fails
        # neuron_isa_check_opcode_on_engine at codegen.
        cv = est["v"] + 60 + (0.521 if packed else 1.042) * fd
        cg = est["g"] + 156 + 1.984 * fd
        if force == "v" or (force is None and cv <= cg):
            est["v"] = cv
            return "v"
        est["g"] = cg
        return "g"

    with tile.TileContext(nc) as tc:
        with ExitStack() as ctx:
            cpool = ctx.enter_context(tc.tile_pool(name="const", bufs=1))
            zpool = ctx.enter_context(tc.tile_pool(name="zp", bufs=1))
            wpool = ctx.enter_context(tc.tile_pool(name="wp", bufs=1))
            zzpool = ctx.enter_context(tc.tile_pool(name="zzp", bufs=2))
            tpool = ctx.enter_context(tc.tile_pool(name="tp", bufs=1))
            mgpool = ctx.enter_context(tc.tile_pool(name="mgp", bufs=2))
            outpool = ctx.enter_context(tc.tile_pool(name="outp", bufs=3))
            pspool = ctx.enter_context(tc.tile_pool(name="ps", bufs=4, space="PSUM"))

            identf = cpool.tile([P, P], f32)
            nc.sync.dma_start(out=identf, in_=ident_d[:, :])
            identP = cpool.tile([P, P], bf16)
            identN = cpool.tile([P, P], bf16)
            nc.scalar.copy(out=identP, in_=identf[:, :])
            nc.scalar.mul(out=identN, in_=identf[:, :], mul=-1.0)
            est["act"] += 2 * (185 + 128 * 0.833)
            idP = identP[:, :]
            idN = identN[:, :]

            warm = pspool.tile([P, WIN * G], f32, tag="ps")
            wrm = warm[:, :]
            nc.tensor.matmul(_ap(wrm, 0, [[1, 1]], bass), idP,
                             _ap(idP, 0, [[1, 1]], bass),
                             start=True, stop=True, skip_group_check=True)
            nc.tensor.matmul(_ap(wrm, 1, [[1, 1]], bass), idN,
                             _ap(idN, 0, [[1, 1]], bass),
                             start=True, stop=True, skip_group_check=True)
            # p-state fillers: keep the PE continuously busy through the
            # z-stage pipeline fill so the first real matmuls run at the
            # ramped 2.4 GHz clock instead of resetting to mid p-state.
            for _f in range(105):
                nc.tensor.matmul(_ap(wrm, 0, [[1, 512]], bass), idP,
                                 _ap(idP, 0, [[1, 512]], bass),
                                 start=True, stop=True, skip_group_check=True)

            def tt(eng, out, in0, in1, op, stt=True):
                if eng == "v":
                    nc.vector.tensor_tensor(out=out, in0=in0, in1=in1, op=op)
                else:
                    nc.gpsimd.tensor_tensor(out=out, in0=in0, in1=in1, op=op)

            cZ, cW, cA = N * G, len(WPAIRS) * G, M * G

            def z_stage_ops(mt):
                """Return (zzT, [closures]) — closures emit the z/w/zz ops."""
                r0 = mt * P * G
                zzT = zzpool.tile([P, 4, M, G], bf16, name=f"zzT{mt}")
                ops = []

                def load():
                    blob = zpool.tile([P, 2, G, N], f32, tag="blob",
                                      name=f"blob{mt}")
                    src_re = z_re_d[r0:r0 + P * G, :].rearrange(
                        "(p g) f -> p g f", g=G)
                    src_im = z_im_d[r0:r0 + P * G, :].rearrange(
                        "(p g) f -> p g f", g=G)
                    nc.sync.dma_start(out=blob[:, 0, :, :], in_=src_re)
                    nc.sync.dma_start(out=blob[:, 1, :, :], in_=src_im)
                    est["dma"] += 2 * 0.46
                    zT = zpool.tile([P, 2, N, G], bf16, tag="zT", name=f"zT{mt}")
                    zb = zT[:, :, :, :]
                    bb = blob[:, :, :, :]
                    nc.scalar.copy(
                        out=_ap(zb, 0, [[cZ, 2], [G, N], [1, G]], bass),
                        in_=_ap(bb, 0, [[G * N, 2], [1, N], [N, G]], bass))
                    est["act"] += 185 + 2 * N * G * 0.833
                    wT = wpool.tile([P, 2, len(WPAIRS), G], bf16, name=f"wT{mt}")
                    state["zb"] = zb
                    state["wbb"] = wT[:, :, :, :]

                state = {}
                ops.append(load)

                def w_step(i):
                    # cross[c1,c2,j] = z[c1,i] * z[c2,i+j]; then
                    # w_re = cross[0,0]-cross[1,1], w_im = cross[1,0]+cross[0,1]
                    zb, wbb = state["zb"], state["wbb"]
                    ti = N - i
                    off = WOFF[(i, i)]
                    t1 = tpool.tile([P, 2, 2, N, G], bf16, tag="ta",
                                    name=f"t1_{mt}_{i}")
                    t1b = t1[:, :, :, :, :]
                    in0 = _ap(zb, i * G, [[cZ, 2], [0, 2], [0, ti], [1, G]], bass)
                    in1 = _ap(zb, i * G, [[0, 2], [cZ, 2], [G, ti], [1, G]], bass)
                    o1 = _ap(t1b, 0,
                             [[2 * N * G, 2], [N * G, 2], [G, ti], [1, G]], bass)
                    tt(pick_tt(4 * ti * G), o1, in0, in1, mult)
                    a0 = _ap(t1b, 0, [[G, ti], [1, G]], bass)
                    a1 = _ap(t1b, 3 * N * G, [[G, ti], [1, G]], bass)
                    ow = _ap(wbb, off * G, [[G, ti], [1, G]], bass)
                    tt(pick_tt(ti * G), ow, a0, a1, sub)
                    a0 = _ap(t1b, 2 * N * G, [[G, ti], [1, G]], bass)
                    a1 = _ap(t1b, N * G, [[G, ti], [1, G]], bass)
                    ow = _ap(wbb, cW + off * G, [[G, ti], [1, G]], bass)
                    tt(pick_tt(ti * G), ow, a0, a1, add)

                for i in range(N):
                    ops.append(lambda i=i: w_step(i))

                ab = zzT[:, :, :, :]

                def zz_step(i, j):
                    # cross[c1,c2,k] = w[c1,(i,j)] * z[c2,j+k]; then
                    # zzR = cross[0,0]-cross[1,1], zzI = cross[1,0]+cross[0,1]
                    zb, wbb = state["zb"], state["wbb"]
                    tk = N - j
                    pr = WOFF[(i, j)]
                    zo = ZOFF[(i, j)]
                    t3 = tpool.tile([P, 2, 2, N, G], bf16, tag="ta",
                                    name=f"t3_{mt}_{pr}")
                    t3b = t3[:, :, :, :, :]
                    in0 = _ap(wbb, pr * G, [[cW, 2], [0, 2], [0, tk], [1, G]], bass)
                    in1 = _ap(zb, j * G, [[0, 2], [cZ, 2], [G, tk], [1, G]], bass)
                    o3 = _ap(t3b, 0,
                             [[2 * N * G, 2], [N * G, 2], [G, tk], [1, G]], bass)
                    tt(pick_tt(4 * tk * G), o3, in0, in1, mult)
                    a0 = _ap(t3b, 0, [[G, tk], [1, G]], bass)
                    a1 = _ap(t3b, 3 * N * G, [[G, tk], [1, G]], bass)
                    oz = _ap(ab, zo * G, [[G, tk], [1, G]], bass)
                    tt(pick_tt(tk * G), oz, a0, a1, sub)
                    a0 = _ap(t3b, 2 * N * G, [[G, tk], [1, G]], bass)
                    a1 = _ap(t3b, N * G, [[G, tk], [1, G]], bass)
                    oz = _ap(ab, cA + zo * G, [[G, tk], [1, G]], bass)
                    tt(pick_tt(tk * G), oz, a0, a1, add)

                for (i, j) in WPAIRS:
                    ops.append(lambda i=i, j=j: zz_step(i, j))

                def sumdif():
                    aR = _ap(ab, 0, [[G, M], [1, G]], bass)
                    aI = _ap(ab, cA, [[G, M], [1, G]], bass)
                    tt(pick_tt(M * G), _ap(ab, 2 * cA, [[G, M], [1, G]], bass),
                       aR, aI, add)
                    tt(pick_tt(M * G), _ap(ab, 3 * cA, [[G, M], [1, G]], bass),
                       aI, aR, sub)

                ops.append(sumdif)
                return zzT, ops

            def products(mt, ab, deferred):
                """Emit product stage for megatile mt; pop some deferred
                z-stage closures (next mt) after each diag."""
                r0 = mt * P * G
                chunk_tiles = {}

                def get_chunk(ci):
                    if ci not in chunk_tiles:
                        occ_t = outpool.tile([P, G, CMAX], bf16, tag="oc",
                                             name=f"oc{mt}_{ci}")
                        chunk_tiles[ci] = occ_t
                    return chunk_tiles[ci]

                writers_left = list(CHUNK_WRITERS)

                def chunk_written(ci):
                    writers_left[ci] -= 1
                    assert writers_left[ci] >= 0
                    if writers_left[ci] == 0:
                        finish_chunk(ci)

                def finish_chunk(ci):
                    cb, ce = CHUNKS[ci]
                    occ = chunk_tiles[ci][:, :, :]
                    dst = out_d[r0:r0 + P * G, cb:ce].rearrange(
                        "(p g) f -> p g f", g=G)
                    nc.sync.dma_start(
                        out=dst, in_=_ap(occ, 0, [[CMAX, G], [1, ce - cb]], bass))
                    est["dma"] += P * G * (ce - cb) * 2 / 360.0

                ps_tiles = {}

                def get_ps(k):
                    if k not in ps_tiles:
                        ps_t = pspool.tile([P, WIN * G], f32, tag="ps",
                                           name=f"ps{mt}_{k}")
                        ps_tiles[k] = ps_t
                    return ps_tiles[k]

                def drain_upto(col):
                    for k, (wa, wz) in enumerate(WINDOWS):
                        if k not in ps_tiles or wz > col:
                            continue
                        psb = ps_tiles.pop(k)[:, :]
                        for ci, (cb, ce) in enumerate(CHUNKS):
                            sa, sz = max(wa, cb), min(wz, ce)
                            if sa >= sz:
                                continue
                            n = sz - sa
                            occ = get_chunk(ci)[:, :, :]
                            src = _ap(psb, (sa - wa) * G, [[G, n], [1, G]], bass)
                            dstc = _ap(occ, sa - cb, [[1, n], [CMAX, G]], bass)
                            nc.scalar.copy(out=dstc, in_=src)
                            est["act"] += 185 + n * G * 0.833
                            chunk_written(ci)

                def pe_accumulate(kind, d, cs, w, mgb):
                    ca = cs
                    while ca < cs + w:
                        k = win_of(ca)
                        wa = WINDOWS[k][0]
                        cb_ = min(ca + 8 - (ca - wa) % 8,
                                  WINDOWS[k][1], cs + w)
                        n = (cb_ - ca) * G
                        psb = get_ps(k)[:, :]
                        pso = _ap(psb, (ca - wa) * G, [[1, n]], bass)
                        t0 = ca - cs
                        m1 = _ap(mgb, t0 * G, [[1, n]], bass)
                        m2 = _ap(mgb, M * G + t0 * G, [[1, n]], bass)
                        if kind == "re":
                            nc.tensor.matmul(pso, idP, m1, start=True, stop=False,
                                             skip_group_check=True)
                            nc.tensor.matmul(pso, idP, m2, start=False, stop=True,
                                             skip_group_check=True)
                            est["pe"] += 2 * n * 0.4167
                        else:
                            m3 = _ap(mgb, 2 * M * G + t0 * G, [[1, n]], bass)
                            nc.tensor.matmul(pso, idP, m3, start=True, stop=False,
                                             skip_group_check=True)
                            nc.tensor.matmul(pso, idP, m1, start=False, stop=False,
                                             skip_group_check=True)
                            nc.tensor.matmul(pso, idN, m2, start=False, stop=True,
                                             skip_group_check=True)
                            est["pe"] += 3 * n * 0.4167
                        ca = cb_

                def gps_add(d, w, mgb):
                    # re_d -> chunk tile(s) on GPSIMD (split at chunk bounds)
                    c0 = RE_COL[d]
                    a = c0
                    while a < c0 + w:
                        ci = _chunk_of(a)
                        cb, ce = CHUNKS[ci]
                        b = min(ce, c0 + w)
                        wl = b - a
                        occ = get_chunk(ci)[:, :, :]
                        og = _ap(occ, a - cb, [[1, wl], [CMAX, G]], bass)
                        m1 = _ap(mgb, (a - c0) * G, [[G, wl], [1, G]], bass)
                        m2 = _ap(mgb, M * G + (a - c0) * G, [[G, wl], [1, G]], bass)
                        nc.gpsimd.tensor_tensor(out=og, in0=m1, in1=m2, op=add)
                        est["g"] += 156 + 1.984 * wl * G
                        chunk_written(ci)
                        a = b

                for d in range(M):
                    w = M - d
                    mg = mgpool.tile([P, 3, M, G], bf16, tag="mg",
                                     name=f"mg{mt}_{d}")
                    mgb = mg[:, :, :, :]
                    in0 = _ap(ab, 0, [[cA, 2], [G, w], [1, G]], bass)
                    in1 = _ap(ab, d * G, [[cA, 2], [G, w], [1, G]], bass)
                    om = _ap(mgb, 0, [[M * G, 2], [G, w], [1, G]], bass)
                    tt(pick_tt(2 * w * G, force="v" if d < K_GPS else None),
                       om, in0, in1, mult)
                    if d >= 1:
                        iD = _ap(ab, 3 * cA, [[G, w], [1, G]], bass)
                        iS = _ap(ab, 2 * cA + d * G, [[G, w], [1, G]], bass)
                        o3 = _ap(mgb, 2 * M * G, [[G, w], [1, G]], bass)
                        tt(pick_tt(w * G, force="v" if d < K_GPS else None),
                           o3, iD, iS, mult)
                    if d < K_GPS:
                        if d >= 1:
                            pe_accumulate("im", d, IM_COL[d], w, mgb)
                        gps_add(d, w, mgb)
                        if d >= 1:
                            drain_upto(IM_COL[d] + w)
                    else:
                        pe_accumulate("re", d, RE_COL[d], w, mgb)
                        pe_accumulate("im", d, IM_COL[d], w, mgb)
                        drain_upto(IM_COL[d] + w)
                    # interleave deferred z-stage emission (next megatile);
                    # hold off until the GPS adds are queued so Pool drains
                    # chunk 0 early
                    if d >= K_GPS:
                        for _ in range(2):
                            if deferred:
                                deferred.pop(0)()
                drain_upto(N_OUT + 1)
                while deferred:
                    deferred.pop(0)()

            zzT0, ops0 = z_stage_ops(0)
            for op in ops0:
                op()
            zzT1, ops1 = z_stage_ops(1)
            products(0, zzT0[:, :, :, :], ops1)
            # boundary fillers: keep the PE ramped across the megatile gap
            warm2 = pspool.tile([P, WIN * G], f32, tag="ps")
            w2b = warm2[:, :]
            for _f in range(30):
                nc.tensor.matmul(_ap(w2b, 0, [[1, 512]], bass), idP,
                                 _ap(idP, 0, [[1, 512]], bass),
                                 start=True, stop=True, skip_group_check=True)
            products(1, zzT1[:, :, :, :], [])

    nc.finalize()
    print(f"[build est us/core] DVE={est['v']/1000:.1f} GPS={est['g']/1000:.1f} "
          f"PE={est['pe']/1000:.1f} ACT={est['act']/1000:.1f} "
          f"DMA={est['dma']/1000:.1f}")
    return nc


_CACHED = {}


def _get_nc():
    if "nc" not in _CACHED:
        _CACHED["nc"] = build_bass()
    return _CACHED["nc"]


def kernel(z_re, z_im):
    from concourse.bass_utils import run_bass_kernel_spmd

    z_re = np.ascontiguousarray(np.asarray(z_re, dtype=np.float32))
    z_im = np.ascontiguousarray(np.asarray(z_im, dtype=np.float32))
    assert z_re.shape == (B_FULL, N), z_re.shape

    nc = _get_nc()
    ident = np.eye(P, dtype=np.float32)
    in_maps = []
    for c in range(NC):
        sl = slice(c * B_LOCAL, (c + 1) * B_LOCAL)
        in_maps.append({
            "z_re": np.ascontiguousarray(z_re[sl]),
            "z_im": np.ascontiguousarray(z_im[sl]),
            "ident": ident,
        })
    res = run_bass_kernel_spmd(nc, in_maps, core_ids=list(range(NC)))
    parts = [np.asarray(res.results[c]["out"]) for c in range(NC)]
    full = np.concatenate(parts, axis=0).astype(np.float32)
    return full[:, PERM]
